# revision 43
# baseline (speedup 1.0000x reference)
"""Trainium2 Bass kernel for nn_EPAN_8735963480604 (sparse_attention).

Reference computation (per batch b, heads h=8, d=64, N=4096, C=512, P=64):
  qkv = x @ w_qkv.T                       -> q,k,v  [H,d,N] views
  k_proj = k @ w_e.T + b_e                -> [H,d,P]   (shared E/F linear)
  v_proj = v @ w_e.T + b_e                -> [H,d,P]
  q = l2normalize(q, axis=N)
  attn = softmax((q^T k_proj) * temperature, axis=P)
  out = attn @ v_proj^T                   -> [H,N,d] -> permute/reshape

Sharding: batch B=8, one batch element per NeuronCore (data parallel),
weights replicated.

Design notes (v3):
 - All matmuls bf16 (1 PE cycle/row, cheap LDWEIGHTS), fp32 PSUM.
 - Host prepares transposed/bf16 layouts (xT, wqT, weT, block-ones J2),
   so no on-chip transposes of x / w_qkv / w_e.
 - k/v never materialized: w_e @ (x @ Wk^T) == (w_e @ x) @ Wk^T; a [P,C]
   "xe" reduction over tokens replaces the whole k|v GEMM.
 - Phase order: q GEMM first (needs only wqT q-columns + xT, so DMA
   prologue is short); the xe token-reduction is interleaved into the q
   loop to absorb DMA slack; projection/softmax setup after; softmax+PV
   last.
 - Heads in pairs g (2x64=128 partitions): block-diagonal stationaries
   give both heads' scores / PV in single matmuls.
 - Softmax denominator via block-ones matmul J2^T @ E on the PE;
   E normalized pre-PV (gpsimd/vector), PV output needs no post-scale.
 - PV emits transposed output [2 heads x d, tokens] == the reference's
   final (d,h,n)-major layout; host reshapes.
"""

import sys

sys.path.insert(0, "/opt/trn_rl_repo")

import numpy as np

N_CORES = 8
B, N, C = 8, 4096, 512
H, D = 8, 64          # heads, head dim
PP = 64               # projection dim P
G = H // 2            # 4 head pairs
CB = C // 128         # 4 c-chunks of 128
NJ = N // 512         # 8 token chunks of 512
NT = N // 128         # 32 token tiles of 128

_cache = {}


def _patch_ldw_opt():
    """Let walrus dedup consecutive LDWEIGHTS of the same stationary."""
    from concourse import bass_utils as bu

    if getattr(bu, "_ldw_patched", False):
        return
    orig = bu.run_command

    def patched(cmd, *a, **kw):
        if isinstance(cmd, list):
            cmd = ["--enable-ldw-opt=true" if c == "--enable-ldw-opt=false"
                   else c for c in cmd]
        return orig(cmd, *a, **kw)

    bu.run_command = patched
    bu._ldw_patched = True


def _build():
    import concourse.bacc as bacc
    import concourse.mybir as mybir
    import concourse.tile as tile

    # note: walrus --enable-ldw-opt=true rejects bass LDWEIGHTS
    # ("not compatible with LDW optimization"), so no dedup is possible.
    f32 = mybir.dt.float32
    bf16 = mybir.dt.bfloat16
    AF = mybir.ActivationFunctionType
    MUL = mybir.AluOpType.mult

    nc = bacc.Bacc("TRN2", target_bir_lowering=False, debug=False,
                   num_devices=N_CORES)

    f8 = mybir.dt.float8e4
    x4_d = nc.dram_tensor("x4", [NJ * 128, 4 * 512], bf16,
                          kind="ExternalInput").ap()
    # partition-major host layouts: [p, cb, .]; q-path in fp8 (w_q scaled
    # x64 on host — q-normalization is scale-invariant)
    xq_d = nc.dram_tensor("xq", [128, CB, N], f8,
                          kind="ExternalInput").ap()
    wqq_d = nc.dram_tensor("wqq", [128, CB, C], f8,
                           kind="ExternalInput").ap()
    wqT_d = nc.dram_tensor("wqkv", [128, CB, 2 * C], bf16,
                           kind="ExternalInput").ap()
    weT_d = nc.dram_tensor("weT", [128, NT * PP], bf16,
                           kind="ExternalInput").ap()
    be_d = nc.dram_tensor("be2", [PP, 1], f32, kind="ExternalInput").ap()
    temp_d = nc.dram_tensor("temp4", [128, G], f32, kind="ExternalInput").ap()
    j2_d = nc.dram_tensor("j2", [128, 128], bf16, kind="ExternalInput").ap()
    id_d = nc.dram_tensor("identb", [128, 128], bf16,
                          kind="ExternalInput").ap()
    ob_d = nc.dram_tensor("ob2", [C, N], bf16, kind="ExternalOutput").ap()
    vposcr_d = nc.dram_tensor("vposcr", [PP, C], bf16, kind="Internal").ap()

    from contextlib import ExitStack

    with tile.TileContext(nc) as tc, ExitStack() as ctx:
        const = ctx.enter_context(tc.tile_pool(name="const", bufs=1))
        xTp = ctx.enter_context(tc.tile_pool(name="xTp", bufs=1))
        wqp = ctx.enter_context(tc.tile_pool(name="wqp", bufs=1))
        x4p = ctx.enter_context(tc.tile_pool(name="x4p", bufs=NJ))
        sbp = ctx.enter_context(tc.tile_pool(name="sbp", bufs=1))
        qTp = ctx.enter_context(tc.tile_pool(name="qTp", bufs=1))
        smallp = ctx.enter_context(tc.tile_pool(name="smallp", bufs=1))
        scrp = ctx.enter_context(tc.tile_pool(name="scrp", bufs=2))
        Ep = ctx.enter_context(tc.tile_pool(name="Ep", bufs=8))
        zrp = ctx.enter_context(tc.tile_pool(name="zrp", bufs=6))
        enp = ctx.enter_context(tc.tile_pool(name="enp", bufs=4))
        otp = ctx.enter_context(tc.tile_pool(name="otp", bufs=4))

        ident = const.tile([128, 128], bf16)
        J2 = const.tile([128, 128], bf16)
        be_sb = const.tile([PP, 1], f32)
        temp_sb = const.tile([128, G], f32)
        weT = const.tile([128, NT * PP], bf16)
        # single tiles with cb as a free-dim axis -> far fewer DMAs
        wqkva = wqp.tile([128, CB, 2 * C], bf16)
        wqqa = wqp.tile([128, CB, C], f8)
        xqa = xTp.tile([128, CB, N], f8)
        x4 = [x4p.tile([128, 4 * 512], bf16, tag="x4", name=f"x4_{j}")
              for j in range(NJ)]

        # DMA order: q-path inputs first (fp8 wq q-cols, then fp8 xT chunk
        # sets per token block), everything else behind them.
        nc.sync.dma_start(wqqa[:], wqq_d[:])
        for j in range(2):
            nc.sync.dma_start(
                xqa[:, :, j * 512:(j + 1) * 512],
                xq_d[:, :, j * 512:(j + 1) * 512],
            )
        nc.sync.dma_start(weT[:], weT_d[:])
        for j in range(2):
            nc.sync.dma_start(x4[j][:], x4_d[j * 128:(j + 1) * 128, :])
        for j in range(2, NJ):
            nc.sync.dma_start(
                xqa[:, :, j * 512:(j + 1) * 512],
                xq_d[:, :, j * 512:(j + 1) * 512],
            )
            nc.sync.dma_start(x4[j][:], x4_d[j * 128:(j + 1) * 128, :])
        nc.sync.dma_start(be_sb[:], be_d[:])
        nc.sync.dma_start(temp_sb[:], temp_d[:])
        nc.sync.dma_start(ident[:], id_d[:])
        nc.sync.dma_start(wqkva[:], wqT_d[:])
        nc.sync.dma_start(J2[:], j2_d[:])

        # persistent small sbuf results
        xe_sb = sbp.tile([PP, C], bf16)          # (w_e @ x)        [p, c]
        xeT_sb = sbp.tile([128, CB, PP], bf16)   # transposed       [c, p]
        kpp_sb = sbp.tile([128, G, PP], bf16)    # k_proj pairs     [hd, g, p]
        vpo_sb = sbp.tile([PP, C], bf16)         # v_proj + b_e     [p, hd]
        vpo2 = [sbp.tile([128, 128], bf16, tag=f"vpo{g}", name=f"vpo{g}")
                for g in range(G)]               # block-diag v_proj [p, hd]
        Sg = [sbp.tile([128, 128], bf16, tag=f"Sg{g}", name=f"Sg{g}")
              for g in range(G)]                 # block-diag scaled k_proj
        qT = [qTp.tile([128, N], bf16, tag=f"qT{g}", name=f"qT{g}")
              for g in range(G)]

        with ExitStack() as ph3:
            ps_q = ph3.enter_context(
                tc.tile_pool(name="ps_q", bufs=3, space="PSUM"))
            ps_xe = ph3.enter_context(
                tc.tile_pool(name="ps_xe", bufs=1, space="PSUM"))
            ps_kv = ph3.enter_context(
                tc.tile_pool(name="ps_kv", bufs=1, space="PSUM"))
            ps_tr = ph3.enter_context(
                tc.tile_pool(name="ps_tr", bufs=2, space="PSUM"))

            # ---- Phase Q: qT[g] = (x @ Wq^T)^T pair rows, bf16; norms;
            #      xe = w_e @ x interleaved (absorbs DMA slack) ----------
            xe_ps = ps_xe.tile([PP, C], f32)
            n2p = [smallp.tile([128, NJ // 2], f32, tag=f"n2p{g}",
                               name=f"n2p{g}")
                   for g in range(G)]
            for j in range(NJ):
                for g in range(G):
                    qp = ps_q.tile([128, 512], f32, tag="qp")
                    for t in range(CB // 2):
                        # fp8 DoubleRow: contracts 2 c-chunks per matmul
                        nc.tensor.matmul(
                            qp[:],
                            wqqa[:, 2 * t:2 * t + 2,
                                 g * 128:(g + 1) * 128],
                            xqa[:, 2 * t:2 * t + 2,
                                j * 512:(j + 1) * 512],
                            start=(t == 0), stop=(t == CB // 2 - 1),
                            perf_mode=mybir.MatmulPerfMode.DoubleRow,
                        )
                    # evict q to bf16 (vector engine)
                    dst = qT[g][:, j * 512:(j + 1) * 512]
                    nc.vector.tensor_copy(dst, qp[:])
                    # pair-wide row-sums of q^2 on the scalar engine
                    if j % 2 == 1:
                        scr = scrp.tile([128, 1024], f32, tag="scr")
                        nc.scalar.activation(
                            scr[:], qT[g][:, (j - 1) * 512:(j + 1) * 512],
                            AF.Square,
                            accum_out=n2p[g][:, j // 2:j // 2 + 1],
                        )
                # xe partial: 4 token tiles of this j-block
                for t in range(4):
                    i = 4 * j + t
                    nc.tensor.matmul(
                        xe_ps[:], weT[:, i * PP:(i + 1) * PP],
                        x4[j][:, t * 512:(t + 1) * 512],
                        start=(i == 0), stop=(i == NT - 1),
                    )

            # ---- projection phase: xeT, kp/vp, vpo2, S setup ------------
            nc.scalar.activation(xe_sb[:], xe_ps[:], AF.Identity)
            for cb in range(CB):
                tp = ps_tr.tile([128, PP], bf16, tag="tp")
                nc.tensor.transpose(
                    tp[:], xe_sb[:, cb * 128:(cb + 1) * 128],
                    ident[0:PP, 0:PP],
                )
                nc.vector.tensor_copy(xeT_sb[:, cb, :], tp[:])

            kpT_ps = ps_kv.tile([PP, C], f32, tag="kpT")
            vpT_ps = ps_kv.tile([PP, C], f32, tag="vpT")
            for cb in range(CB):
                nc.tensor.matmul(
                    kpT_ps[:], xeT_sb[:, cb, :],
                    wqkva[:, cb, 0:C],
                    start=(cb == 0), stop=(cb == CB - 1),
                )
                nc.tensor.matmul(
                    vpT_ps[:], xeT_sb[:, cb, :],
                    wqkva[:, cb, C:2 * C],
                    start=(cb == 0), stop=(cb == CB - 1),
                )
            kpTs = sbp.tile([PP, C], bf16)
            nc.scalar.activation(
                kpTs[:], kpT_ps[:], AF.Identity, bias=be_sb[:, 0:1]
            )
            nc.scalar.activation(
                vpo_sb[:], vpT_ps[:], AF.Identity, bias=be_sb[:, 0:1]
            )
            # vpo2[g]: block-diag [p(A)|p(B), d(A)|d(B)]; the bottom block
            # needs a partition shift -> bounce through DRAM.
            nc.sync.dma_start(vposcr_d[:], vpo_sb[:])
            for g in range(G):
                nc.vector.memset(vpo2[g][:], 0.0)
                nc.sync.dma_start(
                    vpo2[g][0:PP, 0:D],
                    vposcr_d[:, (2 * g) * D:(2 * g + 1) * D],
                )
                nc.sync.dma_start(
                    vpo2[g][PP:128, D:128],
                    vposcr_d[:, (2 * g + 1) * D:(2 * g + 2) * D],
                )
            for g in range(G):
                tp = ps_tr.tile([128, PP], bf16, tag="tp")
                nc.tensor.transpose(
                    tp[:], kpTs[:, g * 128:(g + 1) * 128], ident[0:PP, 0:PP]
                )
                nc.vector.tensor_copy(kpp_sb[:, g, :], tp[:])

            # sv[g] = rsqrt(n2) * temperature; S[g] = blockdiag scaled kp
            for g in range(G):
                n2g = smallp.tile([128, 1], f32, tag=f"n2g{g}",
                                  name=f"n2g{g}")
                nc.vector.reduce_sum(
                    n2g[:], n2p[g][:], mybir.AxisListType.X)
                rc = smallp.tile([128, 1], f32, tag=f"rc{g}", name=f"rc{g}")
                nc.vector.reciprocal(rc[:], n2g[:])
                rs = smallp.tile([128, 1], f32, tag=f"rs{g}", name=f"rs{g}")
                nc.scalar.sqrt(rs[:], rc[:])
                sv = smallp.tile([128, 1], f32, tag=f"sv{g}", name=f"sv{g}")
                nc.vector.tensor_mul(sv[:], rs[:], temp_sb[:, g:g + 1])
                nc.vector.memset(Sg[g][:], 0.0)
                nc.vector.tensor_scalar_mul(
                    Sg[g][0:PP, 0:PP], kpp_sb[0:PP, g, :], sv[0:PP, 0:1]
                )
                nc.vector.tensor_scalar_mul(
                    Sg[g][PP:128, PP:128], kpp_sb[PP:128, g, :],
                    sv[PP:128, 0:1]
                )

        # ---- Phase SM: scores -> softmax -> PV (transposed out) ---------
        with ExitStack() as ph4:
            ps_c = ph4.enter_context(
                tc.tile_pool(name="ps_c", bufs=3, space="PSUM"))
            ps_z = ph4.enter_context(
                tc.tile_pool(name="ps_z", bufs=2, space="PSUM"))
            ps_d = ph4.enter_context(
                tc.tile_pool(name="ps_d", bufs=3, space="PSUM"))
            for g in range(G):
                # E and zr tiles are pair-wide [128, 1024] so the normalize
                # multiply runs as 4 big ops instead of 8.
                E2s = [Ep.tile([128, 1024], bf16, tag="E", name=f"E2_{g}_{p}")
                       for p in range(NJ // 2)]
                zr2s = [zrp.tile([128, 1024], f32, tag="zr",
                                 name=f"zr2_{g}_{p}")
                        for p in range(NJ // 2)]
                for j in range(NJ):
                    cp = ps_c.tile([128, 512], f32, tag="cp")
                    nc.tensor.matmul(
                        cp[:], Sg[g][:], qT[g][:, j * 512:(j + 1) * 512],
                        start=True, stop=True,
                    )
                    E = E2s[j // 2][:, (j % 2) * 512:(j % 2 + 1) * 512]
                    nc.scalar.activation(E, cp[:], AF.Exp)
                for j in range(NJ):
                    zb = ps_z.tile([128, 512], f32, tag="zb")
                    E = E2s[j // 2][:, (j % 2) * 512:(j % 2 + 1) * 512]
                    nc.tensor.matmul(
                        zb[:], J2[:], E, start=True, stop=True,
                    )
                    zr = zr2s[j // 2][:, (j % 2) * 512:(j % 2 + 1) * 512]
                    nc.vector.reciprocal_approx_fast(zr, zb[:])
                ens = []
                for p in range(NJ // 2):
                    en = enp.tile([128, 1024], bf16, tag="en",
                                  name=f"en_{g}_{p}")
                    # last pair-group goes to vector so the final-g drain
                    # finishes in parallel across engines
                    eng = nc.vector if p >= (2 if g == G - 1 else 3) \
                        else nc.gpsimd
                    eng.tensor_tensor(en[:], E2s[p][:], zr2s[p][:], MUL)
                    ens.append(en)
                for j in range(NJ):
                    en = ens[j // 2][:, (j % 2) * 512:(j % 2 + 1) * 512]
                    dp = ps_d.tile([128, 512], f32, tag="dp")
                    nc.tensor.matmul(
                        dp[:], vpo2[g][:], en, start=True, stop=True,
                    )
                    ot = otp.tile([128, 512], bf16, tag="ot")
                    if j % 2 == 0:
                        nc.scalar.copy(ot[:], dp[:])
                    else:
                        nc.vector.tensor_copy(ot[:], dp[:])
                    nc.sync.dma_start(
                        ob_d[g * 128:(g + 1) * 128, j * 512:(j + 1) * 512],
                        ot[:],
                    )

    nc.compile()
    return nc


def _get_nc():
    if "nc" not in _cache:
        _cache["nc"] = _build()
    return _cache["nc"]


def kernel(x, w_qkv, w_e, b_e, temperature):
    import ml_dtypes
    from concourse import bass_utils

    nc = _get_nc()
    bf = ml_dtypes.bfloat16

    x = np.asarray(x, dtype=np.float32)
    w_qkv = np.asarray(w_qkv, dtype=np.float32)
    w_e = np.asarray(w_e, dtype=np.float32)
    b_e2 = np.ascontiguousarray(
        np.asarray(b_e, dtype=np.float32).reshape(PP, 1))
    # temp4[p, g] = temperature[2g + p//64]
    trep = np.repeat(np.asarray(temperature, dtype=np.float32).reshape(H), D)
    temp4 = np.ascontiguousarray(trep.reshape(G, 128).T)

    f8 = ml_dtypes.float8_e4m3
    wqTh = w_qkv.T.reshape(CB, 128, 3 * C).transpose(1, 0, 2)
    # q-columns fp8, scaled x64 (q-normalize is scale-invariant); k/v bf16
    wqq = np.ascontiguousarray(wqTh[:, :, 0:C] * 64.0).astype(f8)
    wqkv = np.ascontiguousarray(wqTh[:, :, C:3 * C]).astype(bf)
    # weT tile layout [128, NT, PP]: weT[p, i, :] = w_e.T[i*128 + p, :]
    weT = np.ascontiguousarray(
        w_e.T.reshape(NT, 128, PP).transpose(1, 0, 2).reshape(128, NT * PP)
    ).astype(bf)
    J2 = np.zeros((128, 128), dtype=np.float32)
    J2[0:PP, 0:PP] = 1.0
    J2[PP:128, PP:128] = 1.0
    J2 = J2.astype(bf)
    identb = np.eye(128, dtype=np.float32).astype(bf)

    in_maps = []
    for c in range(N_CORES):
        xc = x[c]
        # x4 row-block j holds tokens j*512..(j+1)*512 as [128, 4, 512]
        x4 = np.ascontiguousarray(
            xc.reshape(NJ, 4, 128, 512).transpose(0, 2, 1, 3)
        ).reshape(NJ * 128, 4 * 512).astype(bf)
        xq = np.ascontiguousarray(
            xc.T.reshape(CB, 128, N).transpose(1, 0, 2)).astype(f8)
        in_maps.append({
            "x4": x4,
            "xq": xq,
            "wqq": wqq,
            "wqkv": wqkv,
            "weT": weT,
            "be2": b_e2,
            "temp4": temp4,
            "j2": J2,
            "identb": identb,
        })

    import os
    trace = bool(os.environ.get("KERNEL_TRACE"))
    res = bass_utils.run_bass_kernel_spmd(
        nc, in_maps, core_ids=list(range(N_CORES)), trace=trace
    )
    _cache["last_results"] = res

    # per-core ob2 is [C, N] with out[b,h,n,d] at ob2[h*64+d, n].
    # reference returns out.transpose(0,3,1,2).reshape(B,N,C) -> (d,h,n) flat.
    out = np.empty((B, N, C), dtype=np.float32)
    for c in range(N_CORES):
        ob2 = np.asarray(res.results[c]["ob2"], dtype=np.float32)
        out[c] = np.ascontiguousarray(
            ob2.reshape(H, D, N).transpose(1, 0, 2)
        ).reshape(N, C)
    return out
